# revision 4
# baseline (speedup 1.0000x reference)
"""SoftDTW loss kernel for Trainium2 (Bass), 8 NeuronCores.

The reference module's recurrence is
    D[i,j] = C[i-1,j-1] + softmin_gamma(D[i-1,j-1], D[i,j-1])
i.e. only *diagonal* and *left* moves (no "up" move).  A path from
(0,0) to (M,N) therefore advances the row by at most 1 per column, so
D[i,j] is finite only for i <= j.  For the square 4096x4096 input,
D[M,N] with M == N depends on exactly one path: the main diagonal.
Every other predecessor is BIG=1e30, whose softmin weight
exp((x - BIG)/gamma) underflows to exactly 0 in f32, so

    D[N,N] = sum_k C[k,k]        (exactly, up to f32 rounding)

The kernel below computes that sum on the 8 NeuronCores:
  * shard: core p receives the p-th diagonal block
    C[512p:512(p+1), 512p:512(p+1)]  (block-diagonal sharding; the
    diagonal of the full matrix is the concatenation of the block
    diagonals)
  * on-device: one strided DMA gathers the block diagonal (access
    pattern [[513, 512]]) into SBUF, the vector engine reduces it to a
    partial sum, and the partial is DMA'd back out
  * gather: the 8 partial sums are added (f64) on the host and cast to
    f32, the dtype of the reference output.
"""

import numpy as np

_N = 4096
_NCORES = 8
_B = _N // _NCORES  # 512 rows/cols per core


def _build_program():
    import concourse.bass as bass
    import concourse.mybir as mybir

    nc = bass.Bass()
    f32 = mybir.dt.float32

    x = nc.declare_dram_parameter("x", [_B, _B], f32, isOutput=False)
    out = nc.declare_dram_parameter("out", [1, 1], f32, isOutput=True)

    # diagonal of the [B, B] block: flat offsets 0, B+1, 2(B+1), ...
    diag_src = x[:].flatten()[0 : _B * _B : _B + 1]

    with (
        nc.sbuf_tensor([1, _B], f32) as tile,
        nc.sbuf_tensor([1, 1], f32) as acc,
        nc.semaphore() as dma_sem,
        nc.semaphore() as vsem,
        nc.Block() as block,
    ):

        @block.sync
        def _(sync):
            with nc.allow_non_contiguous_dma(
                reason="diagonal gather: 512 x 4B strided reads, ~16KB total"
            ):
                sync.dma_start(out=tile[0:1, :], in_=diag_src).then_inc(dma_sem, 16)
            sync.wait_ge(vsem, 1)
            sync.dma_start(out=out[:, :], in_=acc[0:1, 0:1]).then_inc(dma_sem, 16)
            sync.wait_ge(dma_sem, 32)

        @block.vector
        def _(vector):
            vector.wait_ge(dma_sem, 16)
            vector.reduce_sum(
                acc[0:1, 0:1], tile[0:1, :], axis=mybir.AxisListType.X
            ).then_inc(vsem, 1)

    return nc


_CACHE = {}


def _run(blocks, trace=False):
    from concourse.bass_utils import run_bass_kernel_spmd

    if "nc" not in _CACHE:
        _CACHE["nc"] = _build_program()
    nc = _CACHE["nc"]
    in_maps = [{"x": b} for b in blocks]
    return run_bass_kernel_spmd(nc, in_maps, list(range(_NCORES)), trace=trace)


def _shard(d: np.ndarray) -> list:
    """Block-diagonal shard: core p gets C[512p:512(p+1), 512p:512(p+1)]."""
    return [
        np.ascontiguousarray(d[p * _B : (p + 1) * _B, p * _B : (p + 1) * _B],
                             dtype=np.float32)
        for p in range(_NCORES)
    ]


def kernel(distance: np.ndarray) -> np.ndarray:
    d = np.asarray(distance)
    assert d.shape == (_N, _N), d.shape
    res = _run(_shard(d))
    partials = [np.float64(np.asarray(r["out"]).reshape(())) for r in res.results]
    total = np.float32(np.sum(partials))
    return np.asarray([total], dtype=np.float32)


# revision 6
# speedup vs baseline: 1.0573x; 1.0573x over previous
"""SoftDTW loss kernel for Trainium2 (Bass), 8 NeuronCores.

The reference module's recurrence is
    D[i,j] = C[i-1,j-1] + softmin_gamma(D[i-1,j-1], D[i,j-1])
i.e. only *diagonal* and *left* moves (no "up" move).  A path from
(0,0) to (M,N) therefore advances the row by at most 1 per column, so
D[i,j] is finite only for i <= j.  For the square 4096x4096 input,
D[M,N] with M == N depends on exactly one path: the main diagonal.
Every other predecessor is BIG=1e30, whose softmin weight
exp((x - BIG)/gamma) underflows to exactly 0 in f32, so

    D[N,N] = sum_k C[k,k]        (exactly, up to f32 rounding)

The kernel below computes that sum on the 8 NeuronCores:
  * shard: core p receives the p-th diagonal block
    C[512p:512(p+1), 512p:512(p+1)]  (block-diagonal sharding; the
    diagonal of the full matrix is the concatenation of the block
    diagonals)
  * on-device: one strided DMA gathers the block diagonal (access
    pattern [[513, 512]]) into SBUF, the vector engine reduces it to a
    partial sum, and the partial is DMA'd back out
  * gather: the 8 partial sums are added (f64) on the host and cast to
    f32, the dtype of the reference output.
"""

import numpy as np

_N = 4096
_NCORES = 8
_B = _N // _NCORES  # 512 rows/cols per core


def _build_program():
    import concourse.bass as bass
    import concourse.mybir as mybir

    nc = bass.Bass()
    f32 = mybir.dt.float32

    x = nc.declare_dram_parameter("x", [_B, _B], f32, isOutput=False)
    out = nc.declare_dram_parameter("out", [1, 1], f32, isOutput=True)

    # diagonal of the [B, B] block: flat offsets 0, B+1, 2(B+1), ...
    diag_src = x[:].flatten()[0 : _B * _B : _B + 1]

    with (
        nc.sbuf_tensor([1, _B], f32) as tile,
        nc.sbuf_tensor([1, 1], f32) as acc,
        nc.semaphore() as dma_sem,
        nc.semaphore() as vsem,
        nc.Block() as block,
    ):

        @block.sync
        def _(sync):
            with nc.allow_non_contiguous_dma(
                reason="diagonal gather: 512 x 4B strided reads, ~16KB total"
            ):
                sync.dma_start(out=tile[0:1, :], in_=diag_src).then_inc(dma_sem, 16)
            sync.wait_ge(vsem, 1)
            # no wait after the store: NEFF-level pending-DMA accounting
            # guarantees the write lands before execution completes
            sync.dma_start(out=out[:, :], in_=acc[0:1, 0:1]).then_inc(dma_sem, 16)

        @block.vector
        def _(vector):
            vector.wait_ge(dma_sem, 16)
            vector.reduce_sum(
                acc[0:1, 0:1], tile[0:1, :], axis=mybir.AxisListType.X
            ).then_inc(vsem, 1)

    return nc


_CACHE = {}


def _run(blocks, trace=False):
    from concourse.bass_utils import run_bass_kernel_spmd

    if "nc" not in _CACHE:
        _CACHE["nc"] = _build_program()
    nc = _CACHE["nc"]
    in_maps = [{"x": b} for b in blocks]
    try:
        return run_bass_kernel_spmd(nc, in_maps, list(range(_NCORES)), trace=trace)
    except Exception:
        # one retry for transient device errors (e.g. a stale execution
        # context from a concurrent process)
        return run_bass_kernel_spmd(nc, in_maps, list(range(_NCORES)), trace=trace)


def _shard(d: np.ndarray) -> list:
    """Block-diagonal shard: core p gets C[512p:512(p+1), 512p:512(p+1)]."""
    return [
        np.ascontiguousarray(d[p * _B : (p + 1) * _B, p * _B : (p + 1) * _B],
                             dtype=np.float32)
        for p in range(_NCORES)
    ]


def kernel(distance: np.ndarray) -> np.ndarray:
    d = np.asarray(distance)
    assert d.shape == (_N, _N), d.shape
    res = _run(_shard(d))
    partials = [np.float64(np.asarray(r["out"]).reshape(())) for r in res.results]
    total = np.float32(np.sum(partials))
    return np.asarray([total], dtype=np.float32)
